# revision 35
# baseline (speedup 1.0000x reference)
"""Trainium2 Bass kernel for the AttnModel problem.

Pure data-parallel: batch B=1024 sharded as 128 per core across 8 cores,
small parameters replicated. Key design points:

  - The host concatenates [seq | seq_e | seq_t] feature-wise into a single
    kcat[bpc, N, 3D] tensor and casts it to bf16 (output tolerance is 2e-2;
    bf16 quantization contributes ~5e-3). This halves HBM traffic — the
    dominant cost — and enables the DVE 2-byte packed modes. Weights are
    also cast to bf16 AND pre-transposed on the host, so weight prep is a
    handful of plain row-chunk DMAs (no on-device transposes/copies).
  - kcat is streamed pair-interleaved (partition p holds rows n=2p and
    n=2p+1; each DMA descriptor moves a contiguous 2 rows x 768 feats =
    3KB). Each sub-block tile is split into two DMAs over the two HWDGE
    queues (qSP / qAct), which carry only the k stream plus tiny attn_w
    writebacks; masks and weights ride the Pool/SWDGE queue.
  - sk[n] = kcat[n,:] . wk (softmax over n is shift-invariant, so the
    q-score sq cancels and is never computed). The 25M-MAC score reduction
    is split across three engines, selected per (sub-block, chunk) unit by
    the skpat pattern: 'd' = DVE batched tensor_tensor multiply (2x_1p
    packed) + per-batch tensor_scalar+accum reduce (4x_2p — single-src ops
    keep fast modes with accum_out, two-tensor ops do not); 'p' = GpSimd
    fused multiply+accum; 'a' = DVE multiply + ScalarE (activation Copy)
    accum reduce.
  - softmax is batched over sub-blocks of sb batches using PE transposes
    (partition-dim reductions are not available on VectorE).
  - o[b,:] = attn[b,:] @ k[b] on the TensorEngine with a one-hot-column
    stationary operand (attn column of batch b in slab column b%32, all
    other columns zero, bf16) and kcat streamed as the moving tensor
    (1 cycle/col bf16). Batches of a 32-row cohort accumulate into one
    [32, M] PSUM tile (zero columns contribute zero rows), drained with a
    single quadrant-aligned ScalarE copy — engine SBUF access patterns may
    only start at partition 0/32/64/96, which rules out per-batch row
    placement.
  - The dense chain (fc -> +q residual -> LayerNorm -> agg1+relu -> agg2)
    runs feature-major in bf16 per chainq batches, hidden under the k
    stream; LN statistics over the feature (partition) dim use ones-vector
    matmuls accumulated in fp32 PSUM; the q residual is folded into the fc
    accumulation as identity-rhs matmuls; the LN scale chain runs on
    GpSimd to keep VectorE free. Deep k buffering (kbufs) lets the DMA
    stream run ahead through each chain's PE burst.
"""

import numpy as np
from contextlib import ExitStack

import concourse.bass as bass
import concourse.tile as tile
from concourse import bacc, mybir
from concourse.bass_utils import run_bass_kernel_spmd
from concourse.masks import make_identity

F32 = mybir.dt.float32
BF16 = mybir.dt.bfloat16
U8 = mybir.dt.uint8
AF = mybir.ActivationFunctionType
ALU = mybir.AluOpType
AX = mybir.AxisListType

B, N, D = 1024, 256, 256
M = 3 * D
NCORES = 8
NEG = -1e10
LN_EPS = 1e-5

NCH = 2          # n pair-interleave chunks
MCH = M // 128   # m chunks (6)
DCH = D // 128   # d chunks (2)
FCH = (M + D) // 128  # agg1 input chunks (8)
OCH = D // 128   # output chunks (2)


def build_bass(bpc=B // NCORES, sb=4, skpat="ddda", chain_tok=False,
               repeat=1, kbufs=9, smbufs=5, tpbufs=4, taper=False,
               ln_pool=True, chainq=64, bigb=1, cohb=1,
               pord="A120", lagA=1, lag1=2, lag2=3, chsched=None, wdefer=5,
               chainlag=1):
    """Build the per-core Bass module. bpc = batches per core.

    skpat: engine pattern for sk units, cycled ('d'=DVE, 'p'=GpSimd fused,
    'a'=DVE-mult + ScalarE-reduce).
    chainq: batches per dense-chain invocation (32 or 64).
    chain_tok: add a tok->tok_out passthrough (for serialized timing)."""
    nsb = bpc // sb
    assert nsb * sb == bpc
    assert chainq % 32 == 0
    bp = bpc  # partition count for batch-major tiles

    nc = bacc.Bacc()
    kcat_h = nc.declare_dram_parameter("kcat", [bpc, N, M], BF16, isOutput=False)
    srcb_h = nc.declare_dram_parameter("src_bf", [bpc, D], BF16, isOutput=False)
    srctb_h = nc.declare_dram_parameter("srct_bf", [bpc, D], BF16, isOutput=False)
    srcT_h = nc.declare_dram_parameter("srcT_bf", [D, bpc], BF16, isOutput=False)
    mask_h = nc.declare_dram_parameter("mask", [bpc, N], U8, isOutput=False)
    sha_h = nc.declare_dram_parameter("shared_attn", [1, 2 * M], F32, isOutput=False)
    fcwT_h = nc.declare_dram_parameter("fc_wT", [M, M], BF16, isOutput=False)
    lnw_h = nc.declare_dram_parameter("ln_w", [M], F32, isOutput=False)
    lnb_h = nc.declare_dram_parameter("ln_b", [M], F32, isOutput=False)
    w1T_h = nc.declare_dram_parameter("agg_w1T", [M + D, M], BF16, isOutput=False)
    w2T_h = nc.declare_dram_parameter("agg_w2T", [M, D], BF16, isOutput=False)
    out_h = nc.declare_dram_parameter("out", [bpc, D], F32, isOutput=True)
    attnw_h = nc.declare_dram_parameter("attn_w", [bpc, N], F32, isOutput=True)
    tok_h = tokout_h = None
    if chain_tok:
        tok_h = nc.declare_dram_parameter("tok", [128, 128], F32, isOutput=False)
        tokout_h = nc.declare_dram_parameter("tok_out", [128, 128], F32,
                                             isOutput=True)

    with ExitStack() as ctx:
        tc = ctx.enter_context(tile.TileContext(nc))
        const = ctx.enter_context(tc.tile_pool(name="const", bufs=1))
        kpool = ctx.enter_context(tc.tile_pool(name="kpool", bufs=kbufs))
        jpool = ctx.enter_context(tc.tile_pool(name="jpool", bufs=3))
        sm = ctx.enter_context(tc.tile_pool(name="sm", bufs=smbufs))
        pb = ctx.enter_context(tc.tile_pool(name="pb", bufs=1))
        tp = ctx.enter_context(tc.tile_pool(name="tp", bufs=tpbufs, space="PSUM"))
        bigp = ctx.enter_context(tc.tile_pool(name="bigp", bufs=bigb, space="PSUM"))
        cohp = ctx.enter_context(tc.tile_pool(name="cohp", bufs=cohb, space="PSUM"))

        lne = nc.gpsimd if ln_pool else nc.vector

        # ---------------- constants ----------------
        identity = const.tile([128, 128], F32)
        make_identity(nc, identity)
        identity_bf = const.tile([128, 128], BF16)
        nc.scalar.copy(identity_bf, identity)

        # wk broadcast to all partitions, cast to bf16 (SWDGE casts in-DMA)
        wk_bcast = const.tile([128, M], BF16)
        wk_ap = sha_h[0, M:2 * M]
        nc.gpsimd.dma_start(
            out=wk_bcast,
            in_=bass.AP(tensor=wk_ap.tensor, offset=wk_ap.offset,
                        ap=[[0, 128]] + [list(a) for a in wk_ap.ap]),
        )

        ones_col = const.tile([128, 1], BF16)
        nc.vector.memset(ones_col, 1.0)
        ones_row = const.tile([1, 128], BF16)
        nc.vector.memset(ones_row, 1.0)
        eps_t = const.tile([1, 1], F32)
        nc.vector.memset(eps_t, LN_EPS)

        if chain_tok:
            tok_t = const.tile([128, 128], F32)
            nc.sync.dma_start(out=tok_t, in_=tok_h[:, :])
            nc.sync.dma_start(out=tokout_h[:, :], in_=tok_t)

        # one-hot attn slabs, rotated per sub-block so diag writes for
        # sub-block s never collide with attn@k reads for s-lag:
        # stags[i][:, c, r, v] = attn chunk c of cohort row r at column
        # v==r, zero elsewhere. Zeroed once; only diagonals ever written.
        stags = [const.tile([128, NCH, 32, 32], BF16, name=f"stag{i}")
                 for i in range(4)]
        for st_t in stags:
            nc.scalar.activation(
                out=st_t.rearrange("p c s v -> p (c s v)"),
                in_=identity[:, 0:1].broadcast_to([128, NCH * 32 * 32]),
                func=AF.Copy, scale=0.0)

        # ---------------- weight loads (host pre-transposed bf16) ----------
        lnw_cols = const.tile([128, MCH], F32)
        lnb_cols = const.tile([128, MCH], F32)
        src_rows = const.tile([bp, D], BF16)
        srct_rows = const.tile([bp, D], BF16)
        srcT = const.tile([128, DCH, bp], BF16)
        fc_wT = [const.tile([128, M], BF16, name=f"fcwT{cc}") for cc in range(MCH)]
        w1T = [const.tile([128, M], BF16, name=f"w1T{cc}") for cc in range(FCH)]
        w2T = [const.tile([128, D], BF16, name=f"w2T{cc}") for cc in range(MCH)]

        wq_units = []

        def _pre_dma():
            nc.gpsimd.dma_start(
                out=lnw_cols, in_=lnw_h[:].rearrange("(c p) -> p c", p=128))
            nc.gpsimd.dma_start(
                out=lnb_cols, in_=lnb_h[:].rearrange("(c p) -> p c", p=128))
            nc.gpsimd.dma_start(out=src_rows, in_=srcb_h[:, :])
            nc.gpsimd.dma_start(out=srct_rows, in_=srctb_h[:, :])
            nc.gpsimd.dma_start(
                out=srcT, in_=srcT_h[:, :].rearrange("(dc p) b -> p dc b", p=128))

        wq_units.append(_pre_dma)

        def _w_dma(dst, src_ap):
            def f():
                nc.gpsimd.dma_start(out=dst, in_=src_ap)
            return f

        for cc in range(MCH):
            wq_units.append(_w_dma(fc_wT[cc], fcwT_h[cc * 128:(cc + 1) * 128, :]))
        for fc in range(FCH):
            wq_units.append(_w_dma(w1T[fc], w1T_h[fc * 128:(fc + 1) * 128, :]))
        for ic in range(MCH):
            wq_units.append(_w_dma(w2T[ic], w2T_h[ic * 128:(ic + 1) * 128, :]))

        # ---------------- dense chain (bf16, feature-major) ----------------
        def phase_b_piece1(oT_sb, q0, qn):
            qs = slice(q0, q0 + qn)
            # fcT[j, b] = sum_m fc_w[j, m] oT[m, b]  (+ q residual)
            fc_psum = bigp.tile([128, MCH, qn], F32, tag="big")
            for jc in range(MCH):
                mms = [(fc_wT[mc][:, jc * 128:(jc + 1) * 128], oT_sb[:, mc, qs])
                       for mc in range(MCH)]
                if jc < 2:
                    mms.append((src_rows[:, jc * 128:(jc + 1) * 128],
                                identity_bf[:bp, qs]))
                elif jc >= 4:
                    mms.append((srct_rows[:, (jc - 4) * 128:(jc - 3) * 128],
                                identity_bf[:bp, qs]))
                for q, (l, r) in enumerate(mms):
                    nc.tensor.matmul(fc_psum[:, jc, :], lhsT=l, rhs=r,
                                     start=(q == 0), stop=(q == len(mms) - 1))

            # LayerNorm over the feature (partition) dim
            x_sb = pb.tile([128, MCH, qn], BF16, tag="feat", bufs=3)
            nc.scalar.copy(x_sb.rearrange("p a b -> p (a b)"),
                           fc_psum.rearrange("p a b -> p (a b)"))
            sq_sb = pb.tile([128, MCH, qn], BF16, tag="feat", bufs=3)
            nc.scalar.square(sq_sb.rearrange("p a b -> p (a b)"),
                             fc_psum.rearrange("p a b -> p (a b)"))
            stat = tp.tile([1, 2, qn], F32, tag="tp")
            for mc in range(MCH):
                nc.tensor.matmul(stat[:, 0, :], lhsT=ones_col, rhs=x_sb[:, mc, :],
                                 start=(mc == 0), stop=(mc == MCH - 1))
            for mc in range(MCH):
                nc.tensor.matmul(stat[:, 1, :], lhsT=ones_col, rhs=sq_sb[:, mc, :],
                                 start=(mc == 0), stop=(mc == MCH - 1))

            mu = pb.tile([1, qn], F32, tag="mu", bufs=2)
            nc.vector.tensor_scalar(mu, stat[:, 0, :], 1.0 / M, None, ALU.mult)
            var = pb.tile([1, qn], F32, tag="var", bufs=2)
            nc.vector.tensor_scalar(var, stat[:, 1, :], 1.0 / M, None, ALU.mult)
            musq = pb.tile([1, qn], F32, tag="musq", bufs=2)
            nc.vector.tensor_mul(musq, mu, mu)
            nc.vector.tensor_tensor(var, var, musq, ALU.subtract)
            sd = pb.tile([1, qn], F32, tag="sd", bufs=2)
            nc.scalar.activation(sd, var, AF.Sqrt, bias=eps_t[:, 0:1], scale=1.0)
            ab_row = pb.tile([1, 2, qn], F32, tag="ab", bufs=2)
            nc.vector.reciprocal(ab_row[:, 0, :], sd)                  # rstd
            nc.vector.scalar_tensor_tensor(ab_row[:, 1, :], mu, -1.0,
                                           ab_row[:, 0, :], ALU.mult, ALU.mult)
            ab_bf = pb.tile([1, 2, qn], BF16, tag="abbf", bufs=2)
            nc.scalar.copy(ab_bf.rearrange("p a b -> p (a b)"),
                           ab_row.rearrange("p a b -> p (a b)"))
            bc_ps = tp.tile([128, 2, qn], F32, tag="tp")
            nc.tensor.matmul(bc_ps, lhsT=ones_row,
                             rhs=ab_bf.rearrange("p a b -> p (a b)"),
                             start=True, stop=True)
            bc = pb.tile([128, 2, qn], BF16, tag="bc", bufs=2)
            nc.scalar.copy(bc.rearrange("p a b -> p (a b)"),
                           bc_ps.rearrange("p a b -> p (a b)"))
            return x_sb, bc

        def phase_b_piece2(oT_sb, out_rows2, q0, qn, x_sb, bc):
            qs = slice(q0, q0 + qn)
            xln = pb.tile([128, MCH, qn], BF16, tag="feat", bufs=3)
            for mc in range(MCH):
                lne.tensor_mul(xln[:, mc, :], x_sb[:, mc, :], bc[:, 0, :])
                lne.tensor_add(xln[:, mc, :], xln[:, mc, :], bc[:, 1, :])
                lne.tensor_scalar(xln[:, mc, :], xln[:, mc, :],
                                  lnw_cols[:, mc:mc + 1],
                                  lnb_cols[:, mc:mc + 1], ALU.mult, ALU.add)

            # agg1: x1T[i, b] = relu(sum_f w1[i, f] catT[f, b])
            x1_psum = bigp.tile([128, MCH, qn], F32, tag="big")
            for ic in range(MCH):
                for fc in range(FCH):
                    rhs = xln[:, fc, :] if fc < MCH else srcT[:, fc - MCH, qs]
                    nc.tensor.matmul(x1_psum[:, ic, :],
                                     lhsT=w1T[fc][:, ic * 128:(ic + 1) * 128],
                                     rhs=rhs, start=(fc == 0),
                                     stop=(fc == FCH - 1))
            x1_sb = pb.tile([128, MCH, qn], BF16, tag="feat", bufs=3)
            nc.scalar.activation(x1_sb.rearrange("p a b -> p (a b)"),
                                 x1_psum.rearrange("p a b -> p (a b)"), AF.Relu)

            # agg2: outT[o, b] = sum_i w2[o, i] x1T[i, b]
            outF = bigp.tile([128, OCH, qn], F32, tag="big")
            for oc in range(OCH):
                for ic in range(MCH):
                    nc.tensor.matmul(outF[:, oc, :],
                                     lhsT=w2T[ic][:, oc * 128:(oc + 1) * 128],
                                     rhs=x1_sb[:, ic, :],
                                     start=(ic == 0), stop=(ic == MCH - 1))

            # transpose back to batch-major rows (fp32 output path)
            for oc in range(OCH):
                ot_sb = pb.tile([128, qn], F32, tag="ot_sb", bufs=2)
                nc.scalar.copy(ot_sb, outF[:, oc, :])
                pt3 = tp.tile([qn, 128], F32, name="outtp", tag="tp")
                nc.tensor.transpose(pt3, ot_sb, identity)
                nc.scalar.copy(out_rows2[qs, oc * 128:(oc + 1) * 128], pt3)

        # ---------------- main ----------------
        for _rep in range(repeat):
            oT_sb = pb.tile([128, MCH, bp], BF16, tag="oT", bufs=1)
            out_rows2 = pb.tile([bp, D], F32, tag="or2", bufs=1)
            oCo = None
            pend_b = None
            pend_p1 = None
            wq_next = [0]
            unit_no = [0]

            def emit_w_dma(k=1):
                if _rep == 0:
                    for _ in range(k):
                        if wq_next[0] < len(wq_units):
                            wq_units[wq_next[0]]()
                            wq_next[0] += 1

            def force_weights():
                while wq_next[0] < len(wq_units):
                    emit_w_dma()

            schedule = [(i * sb, sb) for i in range(nsb)]
            if taper and sb % 2 == 0 and nsb >= 2:
                l0 = (nsb - 1) * sb
                schedule = (schedule[:-1]
                            + [(l0, sb // 2), (l0 + sb // 2, sb // 2)])
            nstages = len(schedule)
            state = {}
            # chain trigger edges: batch-count -> chain start. chsched
            # lists per-chain batch counts (e.g. "64,32,32"); default is
            # uniform chainq chunks.
            sizes = ([int(x) for x in str(chsched).split(",")] if chsched
                     else [chainq] * (bpc // chainq))
            assert sum(sizes) == bpc
            ch_edges = {}
            acc = 0
            for sz in sizes:
                ch_edges[acc + sz] = acc
                acc += sz

            def stage0(s):
                b0, sbs = schedule[s]
                # one tile per sub-block, deep-buffered (kbufs): slot-free
                # latency is hidden by depth. Split over both HWDGE queues.
                k_t = kpool.tile([128, sbs, NCH, M], BF16, name="k", tag="k")
                h1 = sbs // 2 if sbs >= 2 else sbs
                src_ap = kcat_h[b0:b0 + sbs].rearrange(
                    "sbs (p c) f -> p sbs c f", p=128)
                nc.sync.dma_start(out=k_t[:, 0:h1], in_=src_ap[:, 0:h1])
                if h1 < sbs:
                    nc.scalar.dma_start(out=k_t[:, h1:sbs], in_=src_ap[:, h1:sbs])
                # weight DMAs deferred past the k-prefill window so they
                # don't steal DMA bandwidth while the pipeline fills
                if s >= wdefer:
                    emit_w_dma(3)

                # masks batched per 32-batch group on the SWDGE queue
                if not taper and sbs == sb and sb <= 32 and 32 % sb == 0:
                    g0 = (b0 // 32) * 32
                    gcnt = min(g0 + 32, bpc) - g0
                    gj = (b0 - g0) // sb
                    if b0 % 32 == 0:
                        mask_g = sm.tile([sb, gcnt // sb, N], U8, tag="mask_g",
                                         bufs=2)
                        nc.gpsimd.dma_start(
                            out=mask_g,
                            in_=mask_h[g0:g0 + gcnt].rearrange(
                                "(j b) n -> b j n", b=sb))
                        state["mask_g"] = mask_g
                    mask_in = state["mask_g"][0:sbs, gj, :]
                else:
                    mask_s = sm.tile([sbs, N], U8, tag="mask_s", bufs=3)
                    nc.gpsimd.dma_start(out=mask_s, in_=mask_h[b0:b0 + sbs, :])
                    mask_in = mask_s
                m_neg = sm.tile([sbs, N], F32, tag="m_neg", bufs=3)
                lne.tensor_scalar(m_neg, mask_in, float(NEG), None, ALU.mult)

                # sk[n] = k[n, :] . wk — engine-split per (sub-block, chunk)
                # unit according to skpat
                sk_st = [sm.tile([128, sbs], F32, name=f"sk{c}", tag=f"sk{c}",
                                 bufs=2)
                         for c in range(NCH)]
                for c in range(NCH):
                    eng = skpat[unit_no[0] % len(skpat)]
                    unit_no[0] += 1
                    jb = jpool.tile([128, sbs, M], BF16, name="jb", tag="jb",
                                    bufs=(3 if sb <= 4 else 2))
                    wk_b = wk_bcast[:, :]
                    wk3 = bass.AP(tensor=wk_b.tensor, offset=wk_b.offset,
                                  ap=[list(wk_b.ap[0]), [0, sbs],
                                      list(wk_b.ap[1])])
                    if eng == "q":
                        # multiply on GpSimd (TensorTensor is Pool-legal)
                        nc.gpsimd.tensor_tensor(jb, k_t[:, :, c, :], wk3,
                                                ALU.mult)
                    else:
                        # DVE batched multiply (2x_1p packed)
                        nc.vector.tensor_tensor(jb, k_t[:, :, c, :], wk3,
                                                ALU.mult)
                    for bi in range(sbs):
                        # 'a': reduces on ScalarE; 'h': alternate; else DVE
                        on_act = (eng == "a"
                                  or (eng == "h" and bi % 2 == 0))
                        if on_act:
                            # reduce on ScalarE (activation Copy + accum)
                            jg2 = jpool.tile([128, 1], BF16, name="junk_a",
                                             tag="junk_a", bufs=2)
                            nc.scalar.activation(
                                out=jg2.broadcast_to([128, M]),
                                in_=jb[:, bi, :], func=AF.Copy,
                                accum_out=sk_st[c][:, bi:bi + 1])
                        else:
                            # reduce on DVE (tensor_scalar+accum keeps 4x_2p)
                            jv2 = jpool.tile([128, M], BF16, name="junk_v",
                                             tag="junk_v", bufs=2)
                            # op1 is the accum reduce op when accum_out is set
                            nc.vector.tensor_scalar(
                                jv2, jb[:, bi, :], 1.0, None, ALU.mult,
                                ALU.add, sk_st[c][:, bi:bi + 1])

                state[("k", s)] = k_t
                state[("sk", s)] = sk_st
                state[("mn", s)] = m_neg

            def stageA(s):
                b0, sbs = schedule[s]
                sk_st = state.pop(("sk", s))
                m_neg = state.pop(("mn", s))
                # transpose sk to batch-major rows, add mask penalty
                s_rows = sm.tile([sbs, N], F32, tag="s_rows", bufs=3)
                s_flat = s_rows[:, :]
                m_flat = m_neg[:, :]
                for c in range(NCH):
                    pt = tp.tile([sbs, 128], F32, name="sktp", tag="tp")
                    nc.tensor.transpose(pt, sk_st[c], identity)
                    # pair-interleaved n: chunk c covers n = 2j + c
                    nc.vector.scalar_tensor_tensor(
                        out=s_flat[:, c:c + 127 * NCH + 1:NCH], in0=pt,
                        scalar=1.0,
                        in1=m_flat[:, c:c + 127 * NCH + 1:NCH],
                        op0=ALU.mult, op1=ALU.add)

                # softmax over free dim. Scores are bounded (|sk| < ~6, and
                # masked entries are -1e10 whose exp underflows to exactly
                # 0), so the usual max-subtraction is unnecessary.
                p_rows = sm.tile([sbs, N], F32, tag="p_rows", bufs=3)
                rsum = sm.tile([sbs, 1], F32, tag="rsum")
                nc.scalar.activation(out=p_rows, in_=s_rows, func=AF.Exp,
                                     scale=1.0, accum_out=rsum)
                rinv = sm.tile([sbs, 1], F32, tag="rinv")
                nc.vector.reciprocal(rinv, rsum)
                a_rows = sm.tile([sbs, N], F32, tag="a_rows", bufs=4)
                lne.tensor_scalar(a_rows, p_rows, rinv[:, 0:1], None, ALU.mult)
                nc.scalar.dma_start(out=attnw_h[b0:b0 + sbs, :], in_=a_rows)
                state[("a", s)] = a_rows

            def stage1(s):
                b0, sbs = schedule[s]
                a_rows = state.pop(("a", s))
                row0 = b0 - (b0 // 32) * 32
                stag = stags[s % len(stags)]
                # attn columns into the one-hot slab diagonals
                for c in range(NCH):
                    pt2 = tp.tile([128, sbs], F32, name="attp", tag="tp")
                    nc.tensor.transpose(
                        pt2, a_rows[:, c:c + 127 * NCH + 1:NCH],
                        identity[:sbs, :sbs])
                    flat = stag[:, c].rearrange("p s v -> p (s v)")
                    nc.scalar.copy(
                        flat[:, row0 * 33:row0 * 33 + (sbs - 1) * 33 + 1:33],
                        pt2[:, 0:sbs])

            def stage2(s):
                nonlocal oCo, co_b0, co_cnt, pend_b, pend_p1
                b0, sbs = schedule[s]
                k_t = state.pop(("k", s))
                stag = stags[s % len(stags)]
                # start of a 32-batch cohort: fresh PSUM accumulator
                if b0 % 32 == 0:
                    oCo = cohp.tile([32, M], F32, tag="coh")
                    co_b0 = b0
                    co_cnt = min(co_b0 + 32, bpc) - co_b0
                row0 = b0 - co_b0
                if pend_b is not None and row0 == 0:
                    # previous chain's second half, one cohort late
                    q0p, qnp, x_sbp, bcp = pend_b
                    phase_b_piece2(oT_sb, out_rows2, q0p, qnp, x_sbp, bcp)
                    pend_b = None
                if pend_p1 is not None:
                    # chain first half, one sub-block after its cohort
                    # drained (keeps its DVE/ACT ops off the drain burst)
                    q0p, qnp = pend_p1
                    if _rep == 0:
                        force_weights()
                    st = phase_b_piece1(oT_sb, q0p, qnp)
                    pend_b = (q0p, qnp) + st
                    pend_p1 = None
                # o[b,:] = attn[b,:] @ k[b]: one-hot attn slab stationary,
                # kcat streamed as moving bf16. Each batch adds its row to
                # the cohort accumulator (other rows get += 0). Split at
                # feature 512 to stay within one PSUM bank per matmul.
                for j in range(sbs):
                    r = row0 + j
                    for h0, hw in ((0, 512), (512, 256)):
                        for c in range(NCH):
                            nc.tensor.matmul(
                                oCo[0:32, h0:h0 + hw],
                                lhsT=stag[:, c, r, :],
                                rhs=k_t[:, j, c, h0:h0 + hw],
                                start=(r == 0 and c == 0),
                                stop=(r == co_cnt - 1 and c == NCH - 1))
                # cohort complete: drain (+ dense chain per chainq batches)
                if b0 + sbs == co_b0 + co_cnt:
                    grp_rows = pb.tile([32, M], BF16, tag="grows", bufs=2)
                    nc.scalar.copy(grp_rows[0:co_cnt, :], oCo[0:co_cnt, :])
                    # cohort oT: batch rows -> feature-major columns
                    for mc in range(MCH):
                        pt = tp.tile([128, co_cnt], BF16, name="otp", tag="tp")
                        nc.tensor.transpose(
                            pt,
                            grp_rows[0:co_cnt, mc * 128:(mc + 1) * 128],
                            identity_bf[:co_cnt, :co_cnt])
                        nc.scalar.copy(oT_sb[:, mc, co_b0:co_b0 + co_cnt], pt)
                    if co_b0 + co_cnt in ch_edges:
                        q0 = ch_edges[co_b0 + co_cnt]
                        if chainlag:
                            pend_p1 = (q0, co_b0 + co_cnt - q0)
                        else:
                            if _rep == 0:
                                force_weights()
                            st = phase_b_piece1(oT_sb, q0,
                                                co_b0 + co_cnt - q0)
                            pend_b = (q0, co_b0 + co_cnt - q0) + st

            co_b0 = co_cnt = 0
            # 4-deep software pipeline. Emission order per iteration puts
            # only data-ready ops at each in-order engine queue's head:
            #   stage0(s):   k DMA + bulk sk ops
            #   stageA(s-lagA): sk transpose + softmax (sk accums ready)
            #   stage1(s-lag1): attn transposes + stag writes (a_rows ready)
            #   stage2(s-lag2): attn@k + cohort drain + chain (stag ready)
            stages = {"0": stage0, "A": stageA, "1": stage1, "2": stage2}
            lags = {"0": 0, "A": lagA, "1": lag1, "2": lag2}
            maxlag = max(lags.values())
            for s in range(nstages + maxlag):
                for ch in pord:
                    lag = lags[ch]
                    if lag <= s < nstages + lag:
                        stages[ch](s - lag)

            if pend_p1 is not None:
                q0p, qnp = pend_p1
                if _rep == 0:
                    force_weights()
                st = phase_b_piece1(oT_sb, q0p, qnp)
                pend_b = (q0p, qnp) + st
                pend_p1 = None
            if pend_b is not None:
                q0p, qnp, x_sbp, bcp = pend_b
                phase_b_piece2(oT_sb, out_rows2, q0p, qnp, x_sbp, bcp)
                pend_b = None
            nc.sync.dma_start(out=out_h[:, :], in_=out_rows2)

    nc.compile()
    return nc


def _shard_inputs(inputs, bpc):
    """Split batch-dim inputs into per-core maps; replicate params.

    All heavy marshalling happens here: seq|seq_e|seq_t concatenated
    feature-wise and cast to bf16 (kcat); weights cast to bf16 and
    pre-transposed so the kernel loads them as plain row chunks."""
    import ml_dtypes

    f32 = lambda x: np.ascontiguousarray(np.asarray(x), dtype=np.float32)
    bf16 = ml_dtypes.bfloat16
    bfc = lambda x: np.ascontiguousarray(np.asarray(x).astype(bf16))
    seq = np.asarray(inputs["seq"])
    seq_e = np.asarray(inputs["seq_e"])
    seq_t = np.asarray(inputs["seq_t"])
    src = np.asarray(inputs["src"])
    src_t = np.asarray(inputs["src_t"])
    mask = np.ascontiguousarray(np.asarray(inputs["mask"])).astype(np.uint8)
    params = {
        "shared_attn": f32(inputs["shared_attn"]),
        "fc_wT": bfc(np.asarray(inputs["fc_w"]).T),
        "ln_w": f32(inputs["ln_w"]),
        "ln_b": f32(inputs["ln_b"]),
        "agg_w1T": bfc(np.asarray(inputs["agg_fc_w1"]).T),
        "agg_w2T": bfc(np.asarray(inputs["agg_fc_w2"]).T),
    }
    in_maps = []
    for i in range(NCORES):
        sl = slice(i * bpc, (i + 1) * bpc)
        kcat = np.empty((bpc, N, M), dtype=bf16)
        kcat[:, :, 0:D] = seq[sl]
        kcat[:, :, D:2 * D] = seq_e[sl]
        kcat[:, :, 2 * D:3 * D] = seq_t[sl]
        src_bf = bfc(src[sl])
        in_maps.append({
            "kcat": kcat,
            "src_bf": src_bf,
            "srct_bf": bfc(src_t[sl, 0, :]),
            "srcT_bf": np.ascontiguousarray(src_bf.T),
            "mask": mask[sl],
            **params,
        })
    return in_maps


_NC_CACHE = []


def kernel(**inputs):
    bpc = B // NCORES
    # build_bass costs seconds of host time; the module is input-independent,
    # so cache it in case the harness calls kernel() repeatedly
    if not _NC_CACHE:
        _NC_CACHE.append(build_bass(bpc=bpc))
    nc = _NC_CACHE[0]
    in_maps = _shard_inputs(inputs, bpc)
    res = run_bass_kernel_spmd(nc, in_maps, core_ids=list(range(NCORES)))
    output = np.concatenate([r["out"] for r in res.results], axis=0)
    attn_w = np.concatenate([r["attn_w"] for r in res.results], axis=0)
    return output, attn_w


# revision 37
# speedup vs baseline: 2.7412x; 2.7412x over previous
"""Trainium2 Bass kernel for the AttnModel problem.

Pure data-parallel: batch B=1024 sharded as 128 per core across 8 cores,
small parameters replicated. Key design points:

  - The host concatenates [seq | seq_e | seq_t] feature-wise into a single
    kcat[bpc, N, 3D] tensor and casts it to bf16 (output tolerance is 2e-2;
    bf16 quantization contributes ~5e-3). This halves HBM traffic — the
    dominant cost — and enables the DVE 2-byte packed modes. Weights are
    also cast to bf16 AND pre-transposed on the host, so weight prep is a
    handful of plain row-chunk DMAs (no on-device transposes/copies).
  - kcat is streamed pair-interleaved (partition p holds rows n=2p and
    n=2p+1; each DMA descriptor moves a contiguous 2 rows x 768 feats =
    3KB). Each sub-block tile is split into two DMAs over the two HWDGE
    queues (qSP / qAct), which carry only the k stream plus tiny attn_w
    writebacks; masks and weights ride the Pool/SWDGE queue.
  - sk[n] = kcat[n,:] . wk (softmax over n is shift-invariant, so the
    q-score sq cancels and is never computed). The 25M-MAC score reduction
    is split across three engines, selected per (sub-block, chunk) unit by
    the skpat pattern: 'd' = DVE batched tensor_tensor multiply (2x_1p
    packed) + per-batch tensor_scalar+accum reduce (4x_2p — single-src ops
    keep fast modes with accum_out, two-tensor ops do not); 'p' = GpSimd
    fused multiply+accum; 'a' = DVE multiply + ScalarE (activation Copy)
    accum reduce.
  - softmax is batched over sub-blocks of sb batches using PE transposes
    (partition-dim reductions are not available on VectorE).
  - o[b,:] = attn[b,:] @ k[b] on the TensorEngine with a one-hot-column
    stationary operand (attn column of batch b in slab column b%32, all
    other columns zero, bf16) and kcat streamed as the moving tensor
    (1 cycle/col bf16). Batches of a 32-row cohort accumulate into one
    [32, M] PSUM tile (zero columns contribute zero rows), drained with a
    single quadrant-aligned ScalarE copy — engine SBUF access patterns may
    only start at partition 0/32/64/96, which rules out per-batch row
    placement.
  - The dense chain (fc -> +q residual -> LayerNorm -> agg1+relu -> agg2)
    runs feature-major in bf16 per chainq batches, hidden under the k
    stream; LN statistics over the feature (partition) dim use ones-vector
    matmuls accumulated in fp32 PSUM; the q residual is folded into the fc
    accumulation as identity-rhs matmuls; the LN scale chain runs on
    GpSimd to keep VectorE free. Deep k buffering (kbufs) lets the DMA
    stream run ahead through each chain's PE burst.
"""

import numpy as np
from contextlib import ExitStack

import concourse.bass as bass
import concourse.tile as tile
from concourse import bacc, mybir
from concourse.bass_utils import run_bass_kernel_spmd
from concourse.masks import make_identity

F32 = mybir.dt.float32
BF16 = mybir.dt.bfloat16
U8 = mybir.dt.uint8
AF = mybir.ActivationFunctionType
ALU = mybir.AluOpType
AX = mybir.AxisListType

B, N, D = 1024, 256, 256
M = 3 * D
NCORES = 8
NEG = -1e10
LN_EPS = 1e-5

NCH = 2          # n pair-interleave chunks
MCH = M // 128   # m chunks (6)
DCH = D // 128   # d chunks (2)
FCH = (M + D) // 128  # agg1 input chunks (8)
OCH = D // 128   # output chunks (2)


def build_bass(bpc=B // NCORES, sb=4, skpat="ffqfaqf", chain_tok=False,
               repeat=1, kbufs=9, smbufs=5, tpbufs=4, taper=False,
               ln_pool=True, chainq=64, bigb=1, cohb=1,
               pord="A210", lagA=1, lag1=2, lag2=3, chsched=None, wdefer=5,
               chainlag=0):
    """Build the per-core Bass module. bpc = batches per core.

    skpat: engine pattern for sk units, cycled ('d'=DVE, 'p'=GpSimd fused,
    'a'=DVE-mult + ScalarE-reduce).
    chainq: batches per dense-chain invocation (32 or 64).
    chain_tok: add a tok->tok_out passthrough (for serialized timing)."""
    nsb = bpc // sb
    assert nsb * sb == bpc
    assert chainq % 32 == 0
    bp = bpc  # partition count for batch-major tiles

    nc = bacc.Bacc()
    kcat_h = nc.declare_dram_parameter("kcat", [bpc, N, M], BF16, isOutput=False)
    srcb_h = nc.declare_dram_parameter("src_bf", [bpc, D], BF16, isOutput=False)
    srctb_h = nc.declare_dram_parameter("srct_bf", [bpc, D], BF16, isOutput=False)
    srcT_h = nc.declare_dram_parameter("srcT_bf", [D, bpc], BF16, isOutput=False)
    mask_h = nc.declare_dram_parameter("mask", [bpc, N], U8, isOutput=False)
    sha_h = nc.declare_dram_parameter("shared_attn", [1, 2 * M], F32, isOutput=False)
    fcwT_h = nc.declare_dram_parameter("fc_wT", [M, M], BF16, isOutput=False)
    lnw_h = nc.declare_dram_parameter("ln_w", [M], F32, isOutput=False)
    lnb_h = nc.declare_dram_parameter("ln_b", [M], F32, isOutput=False)
    w1T_h = nc.declare_dram_parameter("agg_w1T", [M + D, M], BF16, isOutput=False)
    w2T_h = nc.declare_dram_parameter("agg_w2T", [M, D], BF16, isOutput=False)
    out_h = nc.declare_dram_parameter("out", [bpc, D], F32, isOutput=True)
    attnw_h = nc.declare_dram_parameter("attn_w", [bpc, N], F32, isOutput=True)
    tok_h = tokout_h = None
    if chain_tok:
        tok_h = nc.declare_dram_parameter("tok", [128, 128], F32, isOutput=False)
        tokout_h = nc.declare_dram_parameter("tok_out", [128, 128], F32,
                                             isOutput=True)

    with ExitStack() as ctx:
        tc = ctx.enter_context(tile.TileContext(nc))
        const = ctx.enter_context(tc.tile_pool(name="const", bufs=1))
        kpool = ctx.enter_context(tc.tile_pool(name="kpool", bufs=kbufs))
        jpool = ctx.enter_context(tc.tile_pool(name="jpool", bufs=3))
        sm = ctx.enter_context(tc.tile_pool(name="sm", bufs=smbufs))
        pb = ctx.enter_context(tc.tile_pool(name="pb", bufs=1))
        tp = ctx.enter_context(tc.tile_pool(name="tp", bufs=tpbufs, space="PSUM"))
        bigp = ctx.enter_context(tc.tile_pool(name="bigp", bufs=bigb, space="PSUM"))
        cohp = ctx.enter_context(tc.tile_pool(name="cohp", bufs=cohb, space="PSUM"))

        lne = nc.gpsimd if ln_pool else nc.vector

        # ---------------- constants ----------------
        identity = const.tile([128, 128], F32)
        make_identity(nc, identity)
        identity_bf = const.tile([128, 128], BF16)
        nc.scalar.copy(identity_bf, identity)

        # wk broadcast to all partitions, cast to bf16 (SWDGE casts in-DMA)
        wk_bcast = const.tile([128, M], BF16)
        wk_ap = sha_h[0, M:2 * M]
        nc.gpsimd.dma_start(
            out=wk_bcast,
            in_=bass.AP(tensor=wk_ap.tensor, offset=wk_ap.offset,
                        ap=[[0, 128]] + [list(a) for a in wk_ap.ap]),
        )

        ones_col = const.tile([128, 1], BF16)
        nc.vector.memset(ones_col, 1.0)
        ones_row = const.tile([1, 128], BF16)
        nc.vector.memset(ones_row, 1.0)
        eps_t = const.tile([1, 1], F32)
        nc.vector.memset(eps_t, LN_EPS)

        if chain_tok:
            tok_t = const.tile([128, 128], F32)
            nc.sync.dma_start(out=tok_t, in_=tok_h[:, :])
            nc.sync.dma_start(out=tokout_h[:, :], in_=tok_t)

        # one-hot attn slabs, rotated per sub-block so diag writes for
        # sub-block s never collide with attn@k reads for s-lag:
        # stags[i][:, c, r, v] = attn chunk c of cohort row r at column
        # v==r, zero elsewhere. Zeroed once; only diagonals ever written.
        stags = [const.tile([128, NCH, 32, 32], BF16, name=f"stag{i}")
                 for i in range(4)]
        for st_t in stags:
            nc.scalar.activation(
                out=st_t.rearrange("p c s v -> p (c s v)"),
                in_=identity[:, 0:1].broadcast_to([128, NCH * 32 * 32]),
                func=AF.Copy, scale=0.0)

        # ---------------- weight loads (host pre-transposed bf16) ----------
        lnw_cols = const.tile([128, MCH], F32)
        lnb_cols = const.tile([128, MCH], F32)
        src_rows = const.tile([bp, D], BF16)
        srct_rows = const.tile([bp, D], BF16)
        srcT = const.tile([128, DCH, bp], BF16)
        fc_wT = [const.tile([128, M], BF16, name=f"fcwT{cc}") for cc in range(MCH)]
        w1T = [const.tile([128, M], BF16, name=f"w1T{cc}") for cc in range(FCH)]
        w2T = [const.tile([128, D], BF16, name=f"w2T{cc}") for cc in range(MCH)]

        wq_units = []

        def _pre_dma():
            nc.gpsimd.dma_start(
                out=lnw_cols, in_=lnw_h[:].rearrange("(c p) -> p c", p=128))
            nc.gpsimd.dma_start(
                out=lnb_cols, in_=lnb_h[:].rearrange("(c p) -> p c", p=128))
            nc.gpsimd.dma_start(out=src_rows, in_=srcb_h[:, :])
            nc.gpsimd.dma_start(out=srct_rows, in_=srctb_h[:, :])
            nc.gpsimd.dma_start(
                out=srcT, in_=srcT_h[:, :].rearrange("(dc p) b -> p dc b", p=128))

        wq_units.append(_pre_dma)

        def _w_dma(dst, src_ap):
            def f():
                nc.gpsimd.dma_start(out=dst, in_=src_ap)
            return f

        for cc in range(MCH):
            wq_units.append(_w_dma(fc_wT[cc], fcwT_h[cc * 128:(cc + 1) * 128, :]))
        for fc in range(FCH):
            wq_units.append(_w_dma(w1T[fc], w1T_h[fc * 128:(fc + 1) * 128, :]))
        for ic in range(MCH):
            wq_units.append(_w_dma(w2T[ic], w2T_h[ic * 128:(ic + 1) * 128, :]))

        # ---------------- dense chain (bf16, feature-major) ----------------
        def phase_b_piece1(oT_sb, q0, qn):
            qs = slice(q0, q0 + qn)
            # fcT[j, b] = sum_m fc_w[j, m] oT[m, b]  (+ q residual)
            fc_psum = bigp.tile([128, MCH, qn], F32, tag="big")
            for jc in range(MCH):
                mms = [(fc_wT[mc][:, jc * 128:(jc + 1) * 128], oT_sb[:, mc, qs])
                       for mc in range(MCH)]
                if jc < 2:
                    mms.append((src_rows[:, jc * 128:(jc + 1) * 128],
                                identity_bf[:bp, qs]))
                elif jc >= 4:
                    mms.append((srct_rows[:, (jc - 4) * 128:(jc - 3) * 128],
                                identity_bf[:bp, qs]))
                for q, (l, r) in enumerate(mms):
                    nc.tensor.matmul(fc_psum[:, jc, :], lhsT=l, rhs=r,
                                     start=(q == 0), stop=(q == len(mms) - 1))

            # LayerNorm over the feature (partition) dim
            x_sb = pb.tile([128, MCH, qn], BF16, tag="feat", bufs=3)
            nc.scalar.copy(x_sb.rearrange("p a b -> p (a b)"),
                           fc_psum.rearrange("p a b -> p (a b)"))
            sq_sb = pb.tile([128, MCH, qn], BF16, tag="feat", bufs=3)
            nc.scalar.square(sq_sb.rearrange("p a b -> p (a b)"),
                             fc_psum.rearrange("p a b -> p (a b)"))
            stat = tp.tile([1, 2, qn], F32, tag="tp")
            for mc in range(MCH):
                nc.tensor.matmul(stat[:, 0, :], lhsT=ones_col, rhs=x_sb[:, mc, :],
                                 start=(mc == 0), stop=(mc == MCH - 1))
            for mc in range(MCH):
                nc.tensor.matmul(stat[:, 1, :], lhsT=ones_col, rhs=sq_sb[:, mc, :],
                                 start=(mc == 0), stop=(mc == MCH - 1))

            mu = pb.tile([1, qn], F32, tag="mu", bufs=2)
            nc.vector.tensor_scalar(mu, stat[:, 0, :], 1.0 / M, None, ALU.mult)
            var = pb.tile([1, qn], F32, tag="var", bufs=2)
            nc.vector.tensor_scalar(var, stat[:, 1, :], 1.0 / M, None, ALU.mult)
            musq = pb.tile([1, qn], F32, tag="musq", bufs=2)
            nc.vector.tensor_mul(musq, mu, mu)
            nc.vector.tensor_tensor(var, var, musq, ALU.subtract)
            sd = pb.tile([1, qn], F32, tag="sd", bufs=2)
            nc.scalar.activation(sd, var, AF.Sqrt, bias=eps_t[:, 0:1], scale=1.0)
            ab_row = pb.tile([1, 2, qn], F32, tag="ab", bufs=2)
            nc.vector.reciprocal(ab_row[:, 0, :], sd)                  # rstd
            nc.vector.scalar_tensor_tensor(ab_row[:, 1, :], mu, -1.0,
                                           ab_row[:, 0, :], ALU.mult, ALU.mult)
            ab_bf = pb.tile([1, 2, qn], BF16, tag="abbf", bufs=2)
            nc.scalar.copy(ab_bf.rearrange("p a b -> p (a b)"),
                           ab_row.rearrange("p a b -> p (a b)"))
            bc_ps = tp.tile([128, 2, qn], F32, tag="tp")
            nc.tensor.matmul(bc_ps, lhsT=ones_row,
                             rhs=ab_bf.rearrange("p a b -> p (a b)"),
                             start=True, stop=True)
            bc = pb.tile([128, 2, qn], BF16, tag="bc", bufs=2)
            nc.scalar.copy(bc.rearrange("p a b -> p (a b)"),
                           bc_ps.rearrange("p a b -> p (a b)"))
            return x_sb, bc

        def phase_b_piece2(oT_sb, out_rows2, q0, qn, x_sb, bc):
            qs = slice(q0, q0 + qn)
            xln = pb.tile([128, MCH, qn], BF16, tag="feat", bufs=3)
            for mc in range(MCH):
                lne.tensor_mul(xln[:, mc, :], x_sb[:, mc, :], bc[:, 0, :])
                lne.tensor_add(xln[:, mc, :], xln[:, mc, :], bc[:, 1, :])
                lne.tensor_scalar(xln[:, mc, :], xln[:, mc, :],
                                  lnw_cols[:, mc:mc + 1],
                                  lnb_cols[:, mc:mc + 1], ALU.mult, ALU.add)

            # agg1: x1T[i, b] = relu(sum_f w1[i, f] catT[f, b])
            x1_psum = bigp.tile([128, MCH, qn], F32, tag="big")
            for ic in range(MCH):
                for fc in range(FCH):
                    rhs = xln[:, fc, :] if fc < MCH else srcT[:, fc - MCH, qs]
                    nc.tensor.matmul(x1_psum[:, ic, :],
                                     lhsT=w1T[fc][:, ic * 128:(ic + 1) * 128],
                                     rhs=rhs, start=(fc == 0),
                                     stop=(fc == FCH - 1))
            x1_sb = pb.tile([128, MCH, qn], BF16, tag="feat", bufs=3)
            nc.scalar.activation(x1_sb.rearrange("p a b -> p (a b)"),
                                 x1_psum.rearrange("p a b -> p (a b)"), AF.Relu)

            # agg2: outT[o, b] = sum_i w2[o, i] x1T[i, b]
            outF = bigp.tile([128, OCH, qn], F32, tag="big")
            for oc in range(OCH):
                for ic in range(MCH):
                    nc.tensor.matmul(outF[:, oc, :],
                                     lhsT=w2T[ic][:, oc * 128:(oc + 1) * 128],
                                     rhs=x1_sb[:, ic, :],
                                     start=(ic == 0), stop=(ic == MCH - 1))

            # transpose back to batch-major rows (fp32 output path)
            for oc in range(OCH):
                ot_sb = pb.tile([128, qn], F32, tag="ot_sb", bufs=2)
                nc.scalar.copy(ot_sb, outF[:, oc, :])
                pt3 = tp.tile([qn, 128], F32, name="outtp", tag="tp")
                nc.tensor.transpose(pt3, ot_sb, identity)
                nc.scalar.copy(out_rows2[qs, oc * 128:(oc + 1) * 128], pt3)

        # ---------------- main ----------------
        for _rep in range(repeat):
            oT_sb = pb.tile([128, MCH, bp], BF16, tag="oT", bufs=1)
            out_rows2 = pb.tile([bp, D], F32, tag="or2", bufs=1)
            oCo = None
            pend_b = None
            pend_p1 = None
            wq_next = [0]
            unit_no = [0]

            def emit_w_dma(k=1):
                if _rep == 0:
                    for _ in range(k):
                        if wq_next[0] < len(wq_units):
                            wq_units[wq_next[0]]()
                            wq_next[0] += 1

            def force_weights():
                while wq_next[0] < len(wq_units):
                    emit_w_dma()

            schedule = [(i * sb, sb) for i in range(nsb)]
            if taper and sb % 2 == 0 and nsb >= 2:
                l0 = (nsb - 1) * sb
                schedule = (schedule[:-1]
                            + [(l0, sb // 2), (l0 + sb // 2, sb // 2)])
            nstages = len(schedule)
            state = {}
            # chain trigger edges: batch-count -> chain start. chsched
            # lists per-chain batch counts (e.g. "64,32,32"); default is
            # uniform chainq chunks.
            sizes = ([int(x) for x in str(chsched).split(",")] if chsched
                     else [chainq] * (bpc // chainq))
            assert sum(sizes) == bpc
            ch_edges = {}
            acc = 0
            for sz in sizes:
                ch_edges[acc + sz] = acc
                acc += sz

            def stage0(s):
                b0, sbs = schedule[s]
                # one tile per sub-block, deep-buffered (kbufs): slot-free
                # latency is hidden by depth. Split over both HWDGE queues.
                k_t = kpool.tile([128, sbs, NCH, M], BF16, name="k", tag="k")
                h1 = sbs // 2 if sbs >= 2 else sbs
                src_ap = kcat_h[b0:b0 + sbs].rearrange(
                    "sbs (p c) f -> p sbs c f", p=128)
                nc.sync.dma_start(out=k_t[:, 0:h1], in_=src_ap[:, 0:h1])
                if h1 < sbs:
                    nc.scalar.dma_start(out=k_t[:, h1:sbs], in_=src_ap[:, h1:sbs])
                # weight DMAs deferred past the k-prefill window so they
                # don't steal DMA bandwidth while the pipeline fills
                if s >= wdefer:
                    emit_w_dma(3)

                # masks batched per 32-batch group on the SWDGE queue
                if not taper and sbs == sb and sb <= 32 and 32 % sb == 0:
                    g0 = (b0 // 32) * 32
                    gcnt = min(g0 + 32, bpc) - g0
                    gj = (b0 - g0) // sb
                    if b0 % 32 == 0:
                        mask_g = sm.tile([sb, gcnt // sb, N], U8, tag="mask_g",
                                         bufs=2)
                        nc.gpsimd.dma_start(
                            out=mask_g,
                            in_=mask_h[g0:g0 + gcnt].rearrange(
                                "(j b) n -> b j n", b=sb))
                        state["mask_g"] = mask_g
                    mask_in = state["mask_g"][0:sbs, gj, :]
                else:
                    mask_s = sm.tile([sbs, N], U8, tag="mask_s", bufs=3)
                    nc.gpsimd.dma_start(out=mask_s, in_=mask_h[b0:b0 + sbs, :])
                    mask_in = mask_s
                m_neg = sm.tile([sbs, N], F32, tag="m_neg", bufs=3)
                lne.tensor_scalar(m_neg, mask_in, float(NEG), None, ALU.mult)

                # sk[n] = k[n, :] . wk — engine-split per (sub-block, chunk)
                # unit according to skpat
                sk_st = [sm.tile([128, sbs], F32, name=f"sk{c}", tag=f"sk{c}",
                                 bufs=2)
                         for c in range(NCH)]
                for c in range(NCH):
                    eng = skpat[unit_no[0] % len(skpat)]
                    unit_no[0] += 1
                    if eng == "f":
                        # fused multiply+accum on DVE: HW runs ALL accum ops
                        # at 1x (the cost model's fast-mode-with-accum is
                        # wrong on silicon), so one fused op per batch beats
                        # any mult+reduce split kept on DVE
                        for bi in range(sbs):
                            jv = jpool.tile([128, M], BF16, name="junk_v",
                                            tag="junk_v", bufs=2)
                            nc.vector.scalar_tensor_tensor(
                                out=jv, in0=k_t[:, bi, c, :], scalar=1.0,
                                in1=wk_bcast, op0=ALU.mult, op1=ALU.mult,
                                accum_out=sk_st[c][:, bi:bi + 1])
                        continue
                    jb = jpool.tile([128, sbs, M], BF16, name="jb", tag="jb",
                                    bufs=(3 if sb <= 4 else 2))
                    wk_b = wk_bcast[:, :]
                    wk3 = bass.AP(tensor=wk_b.tensor, offset=wk_b.offset,
                                  ap=[list(wk_b.ap[0]), [0, sbs],
                                      list(wk_b.ap[1])])
                    if eng == "q":
                        # multiply on GpSimd (TensorTensor is Pool-legal)
                        nc.gpsimd.tensor_tensor(jb, k_t[:, :, c, :], wk3,
                                                ALU.mult)
                    else:
                        # 'a': DVE batched multiply (2x_1p packed)
                        nc.vector.tensor_tensor(jb, k_t[:, :, c, :], wk3,
                                                ALU.mult)
                    for bi in range(sbs):
                        # reduce on ScalarE (activation Copy + accum)
                        jg2 = jpool.tile([128, 1], BF16, name="junk_a",
                                         tag="junk_a", bufs=2)
                        nc.scalar.activation(
                            out=jg2.broadcast_to([128, M]),
                            in_=jb[:, bi, :], func=AF.Copy,
                            accum_out=sk_st[c][:, bi:bi + 1])

                state[("k", s)] = k_t
                state[("sk", s)] = sk_st
                state[("mn", s)] = m_neg

            def stageA(s):
                b0, sbs = schedule[s]
                sk_st = state.pop(("sk", s))
                m_neg = state.pop(("mn", s))
                # transpose sk to batch-major rows, add mask penalty
                s_rows = sm.tile([sbs, N], F32, tag="s_rows", bufs=3)
                s_flat = s_rows[:, :]
                m_flat = m_neg[:, :]
                for c in range(NCH):
                    pt = tp.tile([sbs, 128], F32, name="sktp", tag="tp")
                    nc.tensor.transpose(pt, sk_st[c], identity)
                    # pair-interleaved n: chunk c covers n = 2j + c
                    nc.vector.scalar_tensor_tensor(
                        out=s_flat[:, c:c + 127 * NCH + 1:NCH], in0=pt,
                        scalar=1.0,
                        in1=m_flat[:, c:c + 127 * NCH + 1:NCH],
                        op0=ALU.mult, op1=ALU.add)

                # softmax over free dim. Scores are bounded (|sk| < ~6, and
                # masked entries are -1e10 whose exp underflows to exactly
                # 0), so the usual max-subtraction is unnecessary.
                p_rows = sm.tile([sbs, N], F32, tag="p_rows", bufs=3)
                rsum = sm.tile([sbs, 1], F32, tag="rsum")
                nc.scalar.activation(out=p_rows, in_=s_rows, func=AF.Exp,
                                     scale=1.0, accum_out=rsum)
                rinv = sm.tile([sbs, 1], F32, tag="rinv")
                nc.vector.reciprocal(rinv, rsum)
                a_rows = sm.tile([sbs, N], F32, tag="a_rows", bufs=4)
                lne.tensor_scalar(a_rows, p_rows, rinv[:, 0:1], None, ALU.mult)
                nc.scalar.dma_start(out=attnw_h[b0:b0 + sbs, :], in_=a_rows)
                state[("a", s)] = a_rows

            def stage1(s):
                b0, sbs = schedule[s]
                a_rows = state.pop(("a", s))
                row0 = b0 - (b0 // 32) * 32
                stag = stags[s % len(stags)]
                # attn columns into the one-hot slab diagonals
                for c in range(NCH):
                    pt2 = tp.tile([128, sbs], F32, name="attp", tag="tp")
                    nc.tensor.transpose(
                        pt2, a_rows[:, c:c + 127 * NCH + 1:NCH],
                        identity[:sbs, :sbs])
                    flat = stag[:, c].rearrange("p s v -> p (s v)")
                    nc.scalar.copy(
                        flat[:, row0 * 33:row0 * 33 + (sbs - 1) * 33 + 1:33],
                        pt2[:, 0:sbs])

            def stage2(s):
                nonlocal oCo, co_b0, co_cnt, pend_b, pend_p1
                b0, sbs = schedule[s]
                k_t = state.pop(("k", s))
                stag = stags[s % len(stags)]
                # start of a 32-batch cohort: fresh PSUM accumulator
                if b0 % 32 == 0:
                    oCo = cohp.tile([32, M], F32, tag="coh")
                    co_b0 = b0
                    co_cnt = min(co_b0 + 32, bpc) - co_b0
                row0 = b0 - co_b0
                if pend_b is not None and row0 == 0:
                    # previous chain's second half, one cohort late
                    q0p, qnp, x_sbp, bcp = pend_b
                    phase_b_piece2(oT_sb, out_rows2, q0p, qnp, x_sbp, bcp)
                    pend_b = None
                if pend_p1 is not None:
                    # chain first half, one sub-block after its cohort
                    # drained (keeps its DVE/ACT ops off the drain burst)
                    q0p, qnp = pend_p1
                    if _rep == 0:
                        force_weights()
                    st = phase_b_piece1(oT_sb, q0p, qnp)
                    pend_b = (q0p, qnp) + st
                    pend_p1 = None
                # o[b,:] = attn[b,:] @ k[b]: one-hot attn slab stationary,
                # kcat streamed as moving bf16. Each batch adds its row to
                # the cohort accumulator (other rows get += 0). Split at
                # feature 512 to stay within one PSUM bank per matmul.
                for j in range(sbs):
                    r = row0 + j
                    for h0, hw in ((0, 512), (512, 256)):
                        for c in range(NCH):
                            nc.tensor.matmul(
                                oCo[0:32, h0:h0 + hw],
                                lhsT=stag[:, c, r, :],
                                rhs=k_t[:, j, c, h0:h0 + hw],
                                start=(r == 0 and c == 0),
                                stop=(r == co_cnt - 1 and c == NCH - 1))
                # cohort complete: drain (+ dense chain per chainq batches)
                if b0 + sbs == co_b0 + co_cnt:
                    grp_rows = pb.tile([32, M], BF16, tag="grows", bufs=2)
                    nc.scalar.copy(grp_rows[0:co_cnt, :], oCo[0:co_cnt, :])
                    # cohort oT: batch rows -> feature-major columns
                    for mc in range(MCH):
                        pt = tp.tile([128, co_cnt], BF16, name="otp", tag="tp")
                        nc.tensor.transpose(
                            pt,
                            grp_rows[0:co_cnt, mc * 128:(mc + 1) * 128],
                            identity_bf[:co_cnt, :co_cnt])
                        nc.scalar.copy(oT_sb[:, mc, co_b0:co_b0 + co_cnt], pt)
                    if co_b0 + co_cnt in ch_edges:
                        q0 = ch_edges[co_b0 + co_cnt]
                        if chainlag:
                            pend_p1 = (q0, co_b0 + co_cnt - q0)
                        else:
                            if _rep == 0:
                                force_weights()
                            st = phase_b_piece1(oT_sb, q0,
                                                co_b0 + co_cnt - q0)
                            pend_b = (q0, co_b0 + co_cnt - q0) + st

            co_b0 = co_cnt = 0
            # 4-deep software pipeline. Emission order per iteration puts
            # only data-ready ops at each in-order engine queue's head:
            #   stage0(s):   k DMA + bulk sk ops
            #   stageA(s-lagA): sk transpose + softmax (sk accums ready)
            #   stage1(s-lag1): attn transposes + stag writes (a_rows ready)
            #   stage2(s-lag2): attn@k + cohort drain + chain (stag ready)
            stages = {"0": stage0, "A": stageA, "1": stage1, "2": stage2}
            lags = {"0": 0, "A": lagA, "1": lag1, "2": lag2}
            maxlag = max(lags.values())
            for s in range(nstages + maxlag):
                for ch in pord:
                    lag = lags[ch]
                    if lag <= s < nstages + lag:
                        stages[ch](s - lag)

            if pend_p1 is not None:
                q0p, qnp = pend_p1
                if _rep == 0:
                    force_weights()
                st = phase_b_piece1(oT_sb, q0p, qnp)
                pend_b = (q0p, qnp) + st
                pend_p1 = None
            if pend_b is not None:
                q0p, qnp, x_sbp, bcp = pend_b
                phase_b_piece2(oT_sb, out_rows2, q0p, qnp, x_sbp, bcp)
                pend_b = None
            nc.sync.dma_start(out=out_h[:, :], in_=out_rows2)

    nc.compile()
    return nc


def _shard_inputs(inputs, bpc):
    """Split batch-dim inputs into per-core maps; replicate params.

    All heavy marshalling happens here: seq|seq_e|seq_t concatenated
    feature-wise and cast to bf16 (kcat); weights cast to bf16 and
    pre-transposed so the kernel loads them as plain row chunks."""
    import ml_dtypes

    f32 = lambda x: np.ascontiguousarray(np.asarray(x), dtype=np.float32)
    bf16 = ml_dtypes.bfloat16
    bfc = lambda x: np.ascontiguousarray(np.asarray(x).astype(bf16))
    seq = np.asarray(inputs["seq"])
    seq_e = np.asarray(inputs["seq_e"])
    seq_t = np.asarray(inputs["seq_t"])
    src = np.asarray(inputs["src"])
    src_t = np.asarray(inputs["src_t"])
    mask = np.ascontiguousarray(np.asarray(inputs["mask"])).astype(np.uint8)
    params = {
        "shared_attn": f32(inputs["shared_attn"]),
        "fc_wT": bfc(np.asarray(inputs["fc_w"]).T),
        "ln_w": f32(inputs["ln_w"]),
        "ln_b": f32(inputs["ln_b"]),
        "agg_w1T": bfc(np.asarray(inputs["agg_fc_w1"]).T),
        "agg_w2T": bfc(np.asarray(inputs["agg_fc_w2"]).T),
    }
    in_maps = []
    for i in range(NCORES):
        sl = slice(i * bpc, (i + 1) * bpc)
        kcat = np.empty((bpc, N, M), dtype=bf16)
        kcat[:, :, 0:D] = seq[sl]
        kcat[:, :, D:2 * D] = seq_e[sl]
        kcat[:, :, 2 * D:3 * D] = seq_t[sl]
        src_bf = bfc(src[sl])
        in_maps.append({
            "kcat": kcat,
            "src_bf": src_bf,
            "srct_bf": bfc(src_t[sl, 0, :]),
            "srcT_bf": np.ascontiguousarray(src_bf.T),
            "mask": mask[sl],
            **params,
        })
    return in_maps


_NC_CACHE = []


def kernel(**inputs):
    bpc = B // NCORES
    # build_bass costs seconds of host time; the module is input-independent,
    # so cache it in case the harness calls kernel() repeatedly
    if not _NC_CACHE:
        _NC_CACHE.append(build_bass(bpc=bpc))
    nc = _NC_CACHE[0]
    in_maps = _shard_inputs(inputs, bpc)
    res = run_bass_kernel_spmd(nc, in_maps, core_ids=list(range(NCORES)))
    output = np.concatenate([r["out"] for r in res.results], axis=0)
    attn_w = np.concatenate([r["attn_w"] for r in res.results], axis=0)
    return output, attn_w
